# revision 1
# baseline (speedup 1.0000x reference)
"""Trainium2 Bass kernel for a conv-attention module.

Computes, for inputs described below (B=8, T1=768, T2=192):
  ke = sepconv(keys);  qe = sepconv chain(queries)        (channel dim NA=64)
  s  = -5e-4 * (|q|^2 + |k|^2 - 2 q.k)                    (B, T1, T2)
  attn_logprob = log_softmax(s, -1) + log(prior + 1e-8)
  attn = softmax(where(mask, -inf, attn_logprob), -1)
Returns (attn, attn_logprob), each (B, 1, T1, T2) float32.

Strategy: data-parallel over batch, one batch element per NeuronCore (8 cores).
All conv params are replicated (packed into two (128, *) f32 tiles).
Per-core layouts are channel-major so every pointwise conv is a PE matmul
with channels on the contraction (partition) axis, and softmax rows (T2)
stay on the free axis.

Numerical notes (validated against the reference on hardware):
  - logits s are in [-0.081, 0], so softmax/logsumexp need no max-subtraction.
  - the attn softmax is computed multiplicatively -- attn = e^s*(prior+1e-8)*
    mask / rowsum(...) -- so no logarithm enters the attn output path; the
    log-softmax shift cancels in the normalization.
  - three ACT table sets are used in strict phases (sigmoid -> exp ->
    natural_log), one load each; natural_log's 40-ULP ln is 10x more accurate
    than the combined exp+ln set's.
  - k=1 depthwise convs and conv biases are folded into the pointwise weights
    and effective biases on the host; the query k=3 conv is folded into the
    contraction dim of the first pointwise matmul (K=45 im2col).
  - |k|^2 rides the attention matmul as an extra lhsT/rhs row pair;
    -5e-4*|q|^2 is applied as a free per-partition ACT bias afterwards.
  - the pad mask multiplies (prior + 1e-8), so masked attn is exactly 0.

Scheduling notes:
  - engine queues execute in order; ops are emitted in expected readiness
    order, interleaving the query and key paths, with a PE warmup burst
    (HAM clock ramp) during the input-DMA window.
  - softmax runs chunk-pipelined over six 128-row blocks through a rotating
    2-slot PSUM pool (per-chunk tiles, so chunks never share a PSUM bank).
  - elementwise work is split across DVE, ACT (per-partition bias/scale),
    and GPSIMD to balance engine load.
"""

import numpy as np

B, T1, T2 = 8, 768, 192
NS, NT, NA = 15, 256, 64
N_CORES = 8
P = 128
IC = T1 // P            # 6 query-row chunks
KC = NT // P            # 2 key-channel chunks
OC = (2 * NT) // P      # 4 chunks of the 512 intermediate key channels
HWD = T1 // 2           # query path processed in 2 halves of 384
CHUNK_ORDER = [0, 1, 2, 3, 4, 5]

_CACHE = {}


class _Cols:
    def __init__(self):
        self.n = 0

    def take(self, ncols):
        s = self.n
        self.n += ncols
        return s


# small weights tile (everything the query path + key depthwise need)
_S = _Cols()
COL_W3Q = _S.take(32)           # q1_dw-fused q1_pw as im2col lhsT (45, 32-pad)
COL_Q2W = _S.take(32)           # q2_dw-folded q2_pw (30, 32-pad)
COL_Q3W = _S.take(64)           # q3_dw-folded q3_pw (15, 64)
COL_KDW = _S.take(2 * 3)        # k1_dw as 2 chunks of (128, 3)
COL_KNPB = _S.take(4)           # -k1_pb_eff, 4 chunks of (128, 1)
COL_KPB = _S.take(4)            # +k1_pb_eff
COL_K2PB = _S.take(1)           # k2_pb_eff (64, 1)
COL_Q1NPB = _S.take(1)          # -q1_pb_eff stacked at rows 0:30 + 32:62
COL_Q1PB = _S.take(1)           # +q1_pb_eff stacked
COL_Q2NPB = _S.take(1)          # -q2_pb_eff stacked at rows 0:15 + 32:47
COL_Q2PB = _S.take(1)           # +q2_pb_eff stacked
COL_Q3PBS = _S.take(1)          # 1e-3 * q3_pb_eff (64, 1)
COL_ONES3 = _S.take(3)          # (64, 3) = [zeros | ones | zeros]
COL_EPS = _S.take(1)            # 1e-8 (128, 1)
COL_BK = _S.take(1)             # rows 64..65 = [1, 0]
COL_BQ = _S.take(1)             # rows 64..65 = [0, -5e-4]
NWS = _S.n

# big weights tile (key pointwise convs)
_Bc = _Cols()
COL_W1K = _Bc.take(2 * 512)     # k1_pw as 2 chunks of (128, 512)
COL_W2K = _Bc.take(4 * 64)      # k2_dw-folded k2_pw as 4 chunks of (128, 64)
NWB = _Bc.n


def _pack_weights(i):
    ws = np.zeros((P, NWS), np.float32)
    wb = np.zeros((P, NWB), np.float32)

    k1_pb_eff = i["k1_pb"] + i["k1_db"] @ i["k1_pw"]              # (512,)
    k2_pw_eff = i["k2_dw"][0, 0][:, None] * i["k2_pw"]            # (512, 64)
    k2_pb_eff = i["k2_pb"] + i["k2_db"] @ k2_pw_eff               # (64,)
    q1_pb_eff = i["q1_pb"] + i["q1_db"] @ i["q1_pw"]              # (30,)
    q2_pw_eff = i["q2_dw"][0, 0][:, None] * i["q2_pw"]            # (30, 15)
    q2_pb_eff = i["q2_pb"] + i["q2_db"] @ q2_pw_eff               # (15,)
    q3_pw_eff = i["q3_dw"][0, 0][:, None] * i["q3_pw"]            # (15, 64)
    q3_pb_eff = i["q3_pb"] + i["q3_db"] @ q3_pw_eff               # (64,)

    for k in range(3):
        ws[15 * k : 15 * k + NS, COL_W3Q : COL_W3Q + 30] = \
            i["q1_dw"][k, 0][:, None] * i["q1_pw"]
    # stage-2/3 lhsT must share the rhs's base partition (0 or 32), so the
    # weights are packed at both row offsets
    for base in (0, 32):
        ws[base : base + 30, COL_Q2W : COL_Q2W + 15] = q2_pw_eff
        ws[base : base + NS, COL_Q3W : COL_Q3W + 64] = q3_pw_eff
    for cc in range(KC):
        ws[:, COL_KDW + 3 * cc : COL_KDW + 3 * (cc + 1)] = \
            i["k1_dw"][:, 0, cc * P : (cc + 1) * P].T
    for c4 in range(OC):
        ws[:, COL_KNPB + c4] = -k1_pb_eff[c4 * P : (c4 + 1) * P]
        ws[:, COL_KPB + c4] = k1_pb_eff[c4 * P : (c4 + 1) * P]
    ws[:NA, COL_K2PB] = k2_pb_eff
    for base in (0, 32):
        ws[base : base + 30, COL_Q1NPB] = -q1_pb_eff
        ws[base : base + 30, COL_Q1PB] = q1_pb_eff
        ws[base : base + NS, COL_Q2NPB] = -q2_pb_eff
        ws[base : base + NS, COL_Q2PB] = q2_pb_eff
    ws[:NA, COL_Q3PBS] = np.float32(1e-3) * q3_pb_eff
    ws[:NA, COL_ONES3 + 1] = 1.0
    ws[:, COL_EPS] = 1e-8
    ws[64, COL_BK] = 1.0
    ws[65, COL_BQ] = -5e-4

    for cc in range(KC):
        wb[:, COL_W1K + 512 * cc : COL_W1K + 512 * (cc + 1)] = \
            i["k1_pw"][cc * P : (cc + 1) * P]
    for c4 in range(OC):
        wb[:, COL_W2K + 64 * c4 : COL_W2K + 64 * (c4 + 1)] = \
            k2_pw_eff[c4 * P : (c4 + 1) * P]
    return ws, wb

def _build():
    import concourse.bass as bass
    import concourse.bacc as bacc
    import concourse.tile as tile
    import concourse.mybir as mybir
    from concourse.hw_specs import get_activation_tables

    f32 = mybir.dt.float32
    ALU = mybir.AluOpType
    AF = mybir.ActivationFunctionType
    X = mybir.AxisListType.X

    nc = bacc.Bacc("TRN2", target_bir_lowering=False, debug=False,
                   enable_asserts=False, num_devices=N_CORES)

    d_qT = nc.dram_tensor("qT", (NS, T1), f32, kind="ExternalInput").ap()
    d_kT = nc.dram_tensor("kT", (NT, T2), f32, kind="ExternalInput").ap()
    d_prior = nc.dram_tensor("prior", (T1, T2), f32, kind="ExternalInput").ap()
    d_am = nc.dram_tensor("am", (1, T2), f32, kind="ExternalInput").ap()
    d_ws = nc.dram_tensor("wts", (P, NWS), f32, kind="ExternalInput").ap()
    d_wb = nc.dram_tensor("wtb", (P, NWB), f32, kind="ExternalInput").ap()
    d_alp = nc.dram_tensor("alp_out", (T1, T2), f32, kind="ExternalOutput").ap()
    d_attn = nc.dram_tensor("attn_out", (T1, T2), f32, kind="ExternalOutput").ap()

    # Engine queues execute in order, so ops are emitted in expected
    # readiness order, interleaving the query and key paths.
    # ACT table sets: sigmoid set (silus) -> exp set (softmax) -> natural_log
    # (the 40-ULP ln for the alp outputs); each boundary is crossed once.
    with tile.TileContext(nc) as tc:
        with tc.tile_pool(name="wp", bufs=1) as wp, \
             tc.tile_pool(name="kp", bufs=1) as kp, \
             tc.tile_pool(name="qp", bufs=1) as qp, \
             tc.tile_pool(name="sm", bufs=1) as sm, \
             tc.tile_pool(name="psum", bufs=1, space="PSUM") as psp, \
             tc.tile_pool(name="psB", bufs=3, space="PSUM") as psB:

            tabs = list(get_activation_tables(nc.m.arch))
            nc.scalar.add_instruction(mybir.InstLoadActFuncSet(
                name=nc.get_next_instruction_name(), ins=[], outs=[],
                act_func_set_id=tabs.index("sigmoid_and_others")))

            # PE warm-up while input DMAs are in flight (HAM clock ramp)
            wrm = wp.tile([P, P], f32)
            nc.vector.memset(wrm, 0.0)
            pwarm = psp.tile([P, P], f32, tag="big")
            for _ in range(6):
                nc.tensor.matmul(pwarm, wrm, wrm, start=True, stop=True)

            # ---- input DMAs, in consumption order ----
            # query im2col over the 3 conv taps: block k holds qT shifted by
            # k-1, so conv1+pointwise is one K=45 matmul per half
            qp3 = qp.tile([45, T1], f32)
            nc.vector.memset(qp3[:, 0:1], 0.0)
            nc.vector.memset(qp3[:, T1 - 1 : T1], 0.0)
            nc.sync.dma_start(out=qp3[0:NS, 1:T1], in_=d_qT[:, 0 : T1 - 1])
            nc.sync.dma_start(out=qp3[NS : 2 * NS, 0:T1], in_=d_qT)
            nc.sync.dma_start(out=qp3[2 * NS : 3 * NS, 0 : T1 - 1],
                              in_=d_qT[:, 1:T1])
            wts = wp.tile([P, NWS], f32)
            nc.sync.dma_start(out=wts, in_=d_ws)
            kpad = kp.tile([P, KC, T2 + 2], f32)
            nc.vector.memset(kpad[:, :, 0:1], 0.0)
            nc.vector.memset(kpad[:, :, T2 + 1 : T2 + 2], 0.0)
            nc.sync.dma_start(out=kpad[:, :, 1 : T2 + 1],
                              in_=d_kT.rearrange("(c p) t -> p c t", p=P))
            wtb = wp.tile([P, NWB], f32)
            nc.sync.dma_start(out=wtb, in_=d_wb)
            pri = sm.tile([P, IC, T2], f32)
            nc.sync.dma_start(
                out=pri, in_=d_prior.rearrange("(c p) j -> p c j", p=P))
            amt = wp.tile([P, T2], f32)
            nc.sync.dma_start(
                out=amt,
                in_=bass.AP(tensor=d_am.tensor, offset=d_am.offset,
                            ap=[[0, P], d_am.ap[1]]))

            # ---- key depthwise conv (k=3): DVE per-tap scale, GPSIMD adds
            m1 = kp.tile([P, KC, T2], f32)
            m0 = kp.tile([P, KC, T2], f32)
            m2 = kp.tile([P, KC, T2], f32)
            for cc in range(KC):
                nc.vector.tensor_scalar_mul(
                    out=m1[:, cc], in0=kpad[:, cc, 1 : T2 + 1],
                    scalar1=wts[:, COL_KDW + 3 * cc + 1 : COL_KDW + 3 * cc + 2])
                nc.vector.tensor_scalar_mul(
                    out=m0[:, cc], in0=kpad[:, cc, 0:T2],
                    scalar1=wts[:, COL_KDW + 3 * cc : COL_KDW + 3 * cc + 1])
                nc.vector.tensor_scalar_mul(
                    out=m2[:, cc], in0=kpad[:, cc, 2 : T2 + 2],
                    scalar1=wts[:, COL_KDW + 3 * cc + 2 : COL_KDW + 3 * cc + 3])
            kda = kp.tile([P, KC, T2], f32)
            nc.vector.tensor_add(out=kda, in0=m1, in1=m0)
            kdf = kp.tile([P, KC, T2], f32)
            nc.vector.tensor_add(out=kdf, in0=kda, in1=m2)

            # ---- query conv1+pointwise 15->30; halves stacked on
            # partitions (base 0 / 32), lhsT padded to M=32
            pq1 = psp.tile([64, 512], f32, tag="q")
            for h in range(2):
                nc.tensor.matmul(
                    pq1[32 * h : 32 * h + 32, 0:HWD],
                    wts[0:45, COL_W3Q : COL_W3Q + 32],
                    qp3[:, h * HWD : (h + 1) * HWD],
                    start=True, stop=True)

            # ---- key pointwise 256 -> 512, output (o, t) channel-major,
            # in two 2-chunk PSUM waves to stay within the bank budget
            pk1h = [psp.tile([P, 2, 512], f32, tag="big", name=f"pk1{w}")
                    for w in range(2)]
            for oc in range(OC):
                for cc in range(KC):
                    nc.tensor.matmul(
                        pk1h[oc // 2][:, oc % 2, 0:T2],
                        wtb[:, COL_W1K + 512 * cc + P * oc :
                               COL_W1K + 512 * cc + P * (oc + 1)],
                        kdf[:, cc],
                        start=(cc == 0), stop=(cc == KC - 1))

            def silu(pool, psum_ap, pb_col, np_, tag):
                """x * sigmoid(x) for x = psum + bias."""
                sg = pool.tile(list(psum_ap.shape), f32, tag=f"{tag}_sg")
                nc.scalar.activation(out=sg, in_=psum_ap, func=AF.Sigmoid,
                                     bias=pb_col[0:np_])
                xb = pool.tile(list(psum_ap.shape), f32, tag=f"{tag}_xb")
                nc.vector.tensor_scalar_add(out=xb, in0=psum_ap,
                                            scalar1=pb_col[0:np_])
                x2 = pool.tile(list(psum_ap.shape), f32, tag=f"{tag}_x2")
                nc.vector.tensor_mul(out=x2, in0=xb, in1=sg)
                return x2

            # ---- query silu 1 + pointwise 30 -> 15 (stacked halves)
            x2q1 = silu(qp, pq1[:, 0:HWD],
                        wts[:, COL_Q1PB : COL_Q1PB + 1], 64, "q1")
            pq2 = psp.tile([64, 512], f32, tag="q")
            for h in range(2):
                nc.tensor.matmul(pq2[32 * h : 32 * h + 32, 0:HWD],
                                 wts[32 * h : 32 * h + 30,
                                     COL_Q2W : COL_Q2W + 32],
                                 x2q1[32 * h : 32 * h + 30, :],
                                 start=True, stop=True)

            # ---- key silu + pointwise 512 -> 64, pipelined per chunk;
            # interleaved with the query stage-2 ops by expected readiness
            sgk = kp.tile([P, OC, T2], f32)
            xbk = kp.tile([P, OC, T2], f32)
            x2k = kp.tile([P, OC, T2], f32)
            pk2 = psp.tile([NA, T2], f32, tag="k2")

            last_sig = []

            def ksilu_oc(oc):
                pk1s = pk1h[oc // 2][:, oc % 2, 0:T2]
                last_sig.append(nc.scalar.activation(
                    out=sgk[:, oc], in_=pk1s, func=AF.Sigmoid,
                    bias=wts[:, COL_KPB + oc : COL_KPB + oc + 1]))
                nc.vector.tensor_scalar_add(
                    out=xbk[:, oc], in0=pk1s,
                    scalar1=wts[:, COL_KPB + oc : COL_KPB + oc + 1])
                nc.vector.tensor_mul(out=x2k[:, oc], in0=xbk[:, oc],
                                     in1=sgk[:, oc])
                nc.tensor.matmul(
                    pk2, wtb[:, COL_W2K + 64 * oc : COL_W2K + 64 * (oc + 1)],
                    x2k[:, oc],
                    start=(oc == 0), stop=(oc == OC - 1))

            ksilu_oc(0)
            ksilu_oc(1)

            # ---- query silu 2 + pointwise 15 -> 64
            x2q2 = silu(qp, pq2[:, 0:HWD],
                        wts[:, COL_Q2PB : COL_Q2PB + 1], 64, "q2")
            pq3 = psp.tile([NA, 2, 512], f32, tag="q")
            for h in range(2):
                nc.tensor.matmul(pq3[:, h, 0:HWD],
                                 wts[32 * h : 32 * h + NS,
                                     COL_Q3W : COL_Q3W + 64],
                                 x2q2[32 * h : 32 * h + NS, :],
                                 start=True, stop=True)

            ksilu_oc(2)
            ksilu_oc(3)

            # switch ACT to the exp table as soon as the last Sigmoid ran
            # (Square/Identity exist in every set, so only Sigmoid pins the
            # sigmoid table); pinned so the scheduler cannot hoist it.
            _eld = mybir.InstLoadActFuncSet(
                name=nc.get_next_instruction_name(), ins=[], outs=[],
                act_func_set_id=tabs.index("exp_and_others"))
            nc.scalar.add_instruction(_eld)
            tile.add_dep_helper(_eld, last_sig[-1].ins, sync=False,
                                reason="exp table after last sigmoid")

            # ---- augmented ke (65, 192): rows 0..63 ke, row 64 = |k|^2
            akt = kp.tile([NA + 1, T2], f32)
            sqk = kp.tile([NA, T2], f32)
            nc.scalar.activation(out=sqk, in_=pk2, func=AF.Square,
                                 bias=wts[0:NA, COL_K2PB : COL_K2PB + 1])
            nc.scalar.activation(out=akt[0:NA, :], in_=pk2, func=AF.Identity,
                                 bias=wts[0:NA, COL_K2PB : COL_K2PB + 1])
            pksq = psB.tile([1, T2], f32, tag="ps", name="pksq")
            nc.tensor.matmul(pksq,
                             wts[0:NA, COL_ONES3 + 1 : COL_ONES3 + 2],
                             sqk, start=True, stop=True)
            nc.vector.tensor_copy(out=akt[NA : NA + 1, :], in_=pksq)

            # ---- augmented qe (65, 768), per half:
            # rows 0..63 = 1e-3*qe, row 64 = -5e-4 (pairs with akt's |k|^2).
            # The -5e-4*|q|^2 term is NOT a matmul row: it is applied later
            # as a free per-partition ACT bias on the e1/s6 passes.
            aq = qp.tile([NA + 1, T1], f32)
            nc.vector.memset(aq[NA : NA + 1, :], -5e-4)
            sqq = qp.tile([NA, T1], f32)
            for h in range(2):
                hs = slice(h * HWD, (h + 1) * HWD)
                nc.scalar.activation(
                    out=sqq[:, hs], in_=pq3[:, h, 0:HWD], func=AF.Square,
                    scale=1e-3, bias=wts[0:NA, COL_Q3PBS : COL_Q3PBS + 1])
                nc.vector.tensor_scalar(
                    out=aq[0:NA, hs], in0=pq3[:, h, 0:HWD],
                    scalar1=1e-3,
                    scalar2=wts[0:NA, COL_Q3PBS : COL_Q3PBS + 1],
                    op0=ALU.mult, op1=ALU.add)
            # per-chunk |q|^2 columns: (128,1) = sqq_chunk.T @ ones
            pqs6 = psB.tile([P, 8], f32, tag="ps", name="pqs6")
            for c in range(IC):
                nc.tensor.matmul(pqs6[:, c : c + 1],
                                 sqq[:, c * P : (c + 1) * P],
                                 wts[0:NA, COL_ONES3 + 1 : COL_ONES3 + 2],
                                 start=True, stop=True)
            qsqc = qp.tile([P, IC], f32)
            nc.vector.tensor_scalar_mul(out=qsqc, in0=pqs6[:, 0:IC],
                                        scalar1=-500.0)

            # pp = prior + 1e-8; pm = pp * mask (0/1).  attn is computed
            # multiplicatively -- attn = e^s*pm / sum(e^s*pm) -- so no
            # logarithm enters the attn path.
            pp = sm.tile([P, IC, T2], f32)
            nc.vector.tensor_scalar_add(out=pp, in0=pri, scalar1=1e-8)
            pm = sm.tile([P, IC, T2], f32)
            for c in range(IC):
                nc.gpsimd.tensor_mul(out=pm[:, c, :], in0=pp[:, c, :],
                                     in1=amt)

            # ---- attention + softmaxes, chunk-pipelined ----
            def mk(nm, shape):
                return [sm.tile(shape, f32, tag=f"{nm}{h}", name=f"{nm}{h}")
                        for h in range(2)]
            z1h = mk("z1", [P, 3])
            lzh = mk("lz", [P, 3])
            z2h = mk("z2", [P, 3])
            r2h = mk("r2", [P, 3])
            oa1 = mk("oa1", [P, 3, T2])
            oa2 = mk("oa2", [P, 3, T2])
            e2t = mk("e2", [P, 3, T2])
            e1t = mk("e1", [P, 3, T2])

            s6 = sm.tile([P, IC, T2], f32)
            for c in CHUNK_ORDER:
                h, col = c // 3, c % 3
                psc = psB.tile([P, T2], f32, tag="ps", name=f"ps{c}")
                nc.tensor.matmul(psc, aq[:, c * P : (c + 1) * P], akt,
                                 start=True, stop=True)
                _e1i = nc.scalar.activation(out=e1t[h][:, col, :], in_=psc,
                                            func=AF.Exp,
                                            bias=qsqc[:, c : c + 1])
                nc.vector.tensor_scalar_add(out=s6[:, c, :], in0=psc,
                                             scalar1=qsqc[:, c : c + 1])
                nc.gpsimd.tensor_mul(out=e2t[h][:, col, :],
                                     in0=e1t[h][:, col, :], in1=pm[:, c, :])
                nc.vector.reduce_sum(out=z2h[h][:, col : col + 1],
                                     in_=e2t[h][:, col, :], axis=X)
                nc.vector.reciprocal(out=r2h[h][:, col : col + 1],
                                     in_=z2h[h][:, col : col + 1])
                nc.vector.tensor_scalar_mul(
                    out=oa2[h][:, col, :], in0=e2t[h][:, col, :],
                    scalar1=r2h[h][:, col : col + 1])

            for h in range(2):
                rows = slice(h * 3 * P, (h + 1) * 3 * P)
                nc.sync.dma_start(
                    out=d_attn[rows, :].rearrange("(c p) j -> p c j", p=P),
                    in_=oa2[h])

            # alp output path: alp = s + log(prior + 1e-8) - log(z1).
            # The Ln ops are pinned after every Exp so the compiler inserts
            # exactly one load of the accurate natural_log table here.
            lp = sm.tile([P, IC, T2], f32)
            t2all = sm.tile([P, IC, T2], f32)
            for h in range(2):
                nc.vector.reduce_sum(out=z1h[h], in_=e1t[h], axis=X)
                _lni = nc.scalar.activation(out=lzh[h], in_=z1h[h],
                                            func=AF.Ln)
                tile.add_dep_helper(_lni.ins, _e1i.ins, sync=False,
                                    reason="ln after all exps (table set)")
                _lni2 = nc.scalar.activation(out=lp[:, 3 * h : 3 * h + 3, :],
                                             in_=pp[:, 3 * h : 3 * h + 3, :],
                                             func=AF.Ln)
                tile.add_dep_helper(_lni2.ins, _e1i.ins, sync=False,
                                    reason="ln after all exps (table set)")
                nc.vector.tensor_add(out=t2all[:, 3 * h : 3 * h + 3, :],
                                     in0=s6[:, 3 * h : 3 * h + 3, :],
                                     in1=lp[:, 3 * h : 3 * h + 3, :])
            for c in CHUNK_ORDER:
                h, col = c // 3, c % 3
                nc.vector.tensor_scalar(
                    out=oa1[h][:, col, :], in0=t2all[:, c, :],
                    scalar1=lzh[h][:, col : col + 1], scalar2=None,
                    op0=ALU.subtract)
            for h in range(2):
                rows = slice(h * 3 * P, (h + 1) * 3 * P)
                nc.sync.dma_start(
                    out=d_alp[rows, :].rearrange("(c p) j -> p c j", p=P),
                    in_=oa1[h])

    nc.finalize()
    return nc


def _get_nc():
    if "nc" not in _CACHE:
        _CACHE["nc"] = _build()
    return _CACHE["nc"]


def kernel(**inputs):
    from concourse.bass_utils import run_bass_kernel_spmd

    i = {k: np.ascontiguousarray(np.asarray(v)) for k, v in inputs.items()}
    ws, wb = _pack_weights(i)

    in_maps = []
    for b in range(N_CORES):
        in_maps.append({
            "qT": np.ascontiguousarray(i["queries"][b].T),
            "kT": np.ascontiguousarray(i["keys"][b].T),
            "prior": np.ascontiguousarray(i["attn_prior"][b]),
            "am": (~i["mask"][b]).astype(np.float32),
            "wts": ws,
            "wtb": wb,
        })

    nc = _get_nc()
    res = run_bass_kernel_spmd(nc, in_maps, core_ids=list(range(N_CORES)),
                               **_CACHE.get("run_kwargs", {}))
    _CACHE["last_result"] = res

    attn = np.stack([r["attn_out"] for r in res.results])[:, None]
    alp = np.stack([r["alp_out"] for r in res.results])[:, None]
    return attn, alp



# revision 10
# speedup vs baseline: 1.4107x; 1.4107x over previous
"""Trainium2 Bass kernel for a conv-attention module.

Computes, for inputs described below (B=8, T1=768, T2=192):
  ke = sepconv(keys);  qe = sepconv chain(queries)        (channel dim NA=64)
  s  = -5e-4 * (|q|^2 + |k|^2 - 2 q.k)                    (B, T1, T2)
  attn_logprob = log_softmax(s, -1) + log(prior + 1e-8)
  attn = softmax(where(mask, -inf, attn_logprob), -1)
Returns (attn, attn_logprob), each (B, 1, T1, T2) float32.

Strategy: data-parallel over batch, one batch element per NeuronCore (8 cores).
All conv params are replicated. Per-core layouts are channel-major so every
pointwise conv is a PE matmul with channels on the contraction (partition)
axis, and softmax rows (T2) stay on the free axis.

Key optimizations over the straightforward implementation:
  - the -5e-4*|q|^2 term is a per-row constant: it cancels exactly in both
    log_softmax and softmax, so it is never computed at all.
  - matmul dtypes: bf16 for all T2-wide (192-col) matmuls (1 cycle/row vs 4
    for fp32), float32r for the 384-col query stage-1 matmul (1 cycle/row at
    free dim >= 256, no cast needed from the f32 DMA).  All elementwise math
    stays f32 except the matmul operand tensors.
  - two ACT table sets total: silu_and_others (native Silu op, fused
    bias+mul) then natural_log_exp_and_others (exp + ln in one set; its ln
    is well within the tolerance).  One mid-kernel switch.
  - Exp runs with accum_out, so the log-softmax denominator z1 is a free
    by-product; e2 = e1*pm and z2 = rowsum(e2) fuse into one
    scalar_tensor_tensor with accum_out; alp = (s - ln z1) + ln(prior+eps)
    is one scalar_tensor_tensor from PSUM.
  - the attn softmax is computed multiplicatively (attn = e^s*pm / rowsum),
    mask folded into pm = (prior+1e-8)*mask01, so masked attn is exactly 0.
  - DMA dispatch is split across the SP and ACT HWDGE queues, the prior
    ships as two halves, and each output half is DMA'd as soon as it is
    finalized.
  - k=1 depthwise convs and conv biases are folded into pointwise weights on
    the host; the key k=3 depthwise conv is a 3-op fused mul-add chain on
    DVE; the query k=3 conv is folded into the contraction of the first
    pointwise matmul (K=45 im2col via 3 shifted DMA views).
"""

import numpy as np

B, T1, T2 = 8, 768, 192
NS, NT, NA = 15, 256, 64
N_CORES = 8
P = 128
IC = T1 // P            # 6 query-row chunks
KC = NT // P            # 2 key-channel chunks
OC = (2 * NT) // P      # 4 chunks of the 512 intermediate key channels
HWD = T1 // 2           # query path processed in 2 halves of 384

_CACHE = {}


class _Cols:
    def __init__(self):
        self.n = 0

    def take(self, ncols):
        s = self.n
        self.n += ncols
        return s


# small f32 weights tile: per-partition scalars and biases
_S = _Cols()
COL_KDW = _S.take(2 * 3)        # k1_dw as 2 chunks of (128, 3)
COL_KPB = _S.take(4)            # k1_pb_eff, 4 chunks of (128, 1)
COL_K2PB = _S.take(1)           # k2_pb_eff (64, 1)
COL_Q1PB = _S.take(1)           # q1_pb_eff stacked at rows 0:30 + 32:62
COL_Q2PB = _S.take(1)           # q2_pb_eff stacked at rows 0:15 + 32:47
COL_Q3PBS = _S.take(1)          # 1e-3 * q3_pb_eff (64, 1)
COL_EPS = _S.take(1)            # 1e-8 (128, 1)
COL_ZERO = _S.take(1)           # 0.0 (128, 1) -- zero activation bias
NWS = _S.n

# big bf16 weights tile (all matmul lhsT blocks except the q1 im2col)
_Bc = _Cols()
COL_W1K = _Bc.take(2 * 512)     # k1_pw as 2 chunks of (128, 512)
COL_W2K = _Bc.take(4 * 64)      # k2_dw-folded k2_pw as 4 chunks of (128, 64)
COL_Q2W = _Bc.take(32)          # q2_dw-folded q2_pw (30, 32-pad), both bases
COL_Q3W = _Bc.take(64)          # q3_dw-folded q3_pw (15, 64), both bases
COL_ONE = _Bc.take(1)           # ones at rows 0:64
COL_Q1W = _Bc.take(32)          # q1_dw-fused q1_pw as im2col lhsT (45, 32-pad)
NWB = _Bc.n


def _pack_weights(i):
    import ml_dtypes

    ws = np.zeros((P, NWS), np.float32)
    wb = np.zeros((P, NWB), np.float32)

    k1_pb_eff = i["k1_pb"] + i["k1_db"] @ i["k1_pw"]              # (512,)
    k2_pw_eff = i["k2_dw"][0, 0][:, None] * i["k2_pw"]            # (512, 64)
    k2_pb_eff = i["k2_pb"] + i["k2_db"] @ k2_pw_eff               # (64,)
    q1_pb_eff = i["q1_pb"] + i["q1_db"] @ i["q1_pw"]              # (30,)
    q2_pw_eff = i["q2_dw"][0, 0][:, None] * i["q2_pw"]            # (30, 15)
    q2_pb_eff = i["q2_pb"] + i["q2_db"] @ q2_pw_eff               # (15,)
    q3_pw_eff = i["q3_dw"][0, 0][:, None] * i["q3_pw"]            # (15, 64)
    q3_pb_eff = i["q3_pb"] + i["q3_db"] @ q3_pw_eff               # (64,)

    for cc in range(KC):
        ws[:, COL_KDW + 3 * cc : COL_KDW + 3 * (cc + 1)] = \
            i["k1_dw"][:, 0, cc * P : (cc + 1) * P].T
    for c4 in range(OC):
        ws[:, COL_KPB + c4] = k1_pb_eff[c4 * P : (c4 + 1) * P]
    ws[:NA, COL_K2PB] = k2_pb_eff
    for base in (0, 32):
        ws[base : base + 30, COL_Q1PB] = q1_pb_eff
        ws[base : base + NS, COL_Q2PB] = q2_pb_eff
    ws[:NA, COL_Q3PBS] = np.float32(1e-3) * q3_pb_eff
    ws[:, COL_EPS] = 1e-8

    for cc in range(KC):
        wb[:, COL_W1K + 512 * cc : COL_W1K + 512 * (cc + 1)] = \
            i["k1_pw"][cc * P : (cc + 1) * P]
    for c4 in range(OC):
        wb[:, COL_W2K + 64 * c4 : COL_W2K + 64 * (c4 + 1)] = \
            k2_pw_eff[c4 * P : (c4 + 1) * P]
    # stage-2/3 lhsT must share the rhs's base partition (0 or 32)
    for base in (0, 32):
        wb[base : base + 30, COL_Q2W : COL_Q2W + 15] = q2_pw_eff
        wb[base : base + NS, COL_Q3W : COL_Q3W + 64] = q3_pw_eff
    wb[:NA, COL_ONE] = 1.0

    for k in range(3):
        wb[15 * k : 15 * k + NS, COL_Q1W : COL_Q1W + 30] = \
            i["q1_dw"][k, 0][:, None] * i["q1_pw"]

    return ws, wb.astype(ml_dtypes.bfloat16)


def _build():
    import concourse.bass as bass
    import concourse.bacc as bacc
    import concourse.tile as tile
    import concourse.mybir as mybir
    from concourse.hw_specs import get_activation_tables

    f32 = mybir.dt.float32
    f32r = mybir.dt.float32r
    bf16 = mybir.dt.bfloat16
    ALU = mybir.AluOpType
    AF = mybir.ActivationFunctionType

    nc = bacc.Bacc("TRN2", target_bir_lowering=False, debug=False,
                   enable_asserts=False, num_devices=N_CORES)

    d_qT = nc.dram_tensor("qT", (NS, T1), f32, kind="ExternalInput").ap()
    d_kT = nc.dram_tensor("kT", (NT, T2), f32, kind="ExternalInput").ap()
    d_prior = nc.dram_tensor("prior", (T1, T2), f32, kind="ExternalInput").ap()
    d_am = nc.dram_tensor("am", (1, T2), f32, kind="ExternalInput").ap()
    d_ws = nc.dram_tensor("wts", (P, NWS), f32, kind="ExternalInput").ap()
    d_wb = nc.dram_tensor("wtb", (P, NWB), bf16, kind="ExternalInput").ap()
    d_alp = nc.dram_tensor("alp_out", (T1, T2), f32, kind="ExternalOutput").ap()
    d_attn = nc.dram_tensor("attn_out", (T1, T2), f32, kind="ExternalOutput").ap()

    with tile.TileContext(nc) as tc:
        with tc.tile_pool(name="wp", bufs=1) as wp, \
             tc.tile_pool(name="kp", bufs=1) as kp, \
             tc.tile_pool(name="qp", bufs=1) as qp, \
             tc.tile_pool(name="sm", bufs=1) as sm, \
             tc.tile_pool(name="psum", bufs=1, space="PSUM") as psp, \
             tc.tile_pool(name="psB", bufs=3, space="PSUM") as psB:

            tabs = list(get_activation_tables(nc.m.arch))

            # ---- input DMAs ----------------------------------------------
            # ACT HWDGE queue: the two small weight tiles, then the silu
            # table load (runs while the SP-queue DMAs stream).
            wts = wp.tile([P, NWS], f32)
            nc.scalar.dma_start(out=wts, in_=d_ws)
            _ld0 = mybir.InstLoadActFuncSet(
                name=nc.get_next_instruction_name(), ins=[], outs=[],
                act_func_set_id=tabs.index("silu_and_others"))
            nc.scalar.add_instruction(_ld0)

            # SP HWDGE queue, in consumption order.
            kpad = kp.tile([P, KC, T2 + 2], f32)
            nc.vector.memset(kpad[:, :, 0:1], 0.0)
            nc.vector.memset(kpad[:, :, T2 + 1 : T2 + 2], 0.0)
            nc.sync.dma_start(out=kpad[:, :, 1 : T2 + 1],
                              in_=d_kT.rearrange("(c p) t -> p c t", p=P))
            wtb = wp.tile([P, NWB], bf16)
            nc.sync.dma_start(out=wtb, in_=d_wb)
            # query im2col over the 3 conv taps: block k holds qT shifted by
            # k-1, so conv1+pointwise is one K=45 matmul per half
            # query im2col tile in bf16: gpsimd (SWDGE) DMAs cast f32->bf16
            # in flight, on their own dispatch queue
            qp3 = qp.tile([45, T1], bf16)
            nc.vector.memset(qp3[:, 0:1], 0.0)
            nc.vector.memset(qp3[:, T1 - 1 : T1], 0.0)
            nc.gpsimd.dma_start(out=qp3[NS : 2 * NS, 0:T1], in_=d_qT)
            nc.gpsimd.dma_start(out=qp3[0:NS, 1:T1], in_=d_qT[:, 0 : T1 - 1])
            nc.gpsimd.dma_start(out=qp3[2 * NS : 3 * NS, 0 : T1 - 1],
                              in_=d_qT[:, 1:T1])
            pri = sm.tile([P, IC, T2], f32)
            nc.sync.dma_start(
                out=pri[:, 0:3],
                in_=d_prior[0:HWD].rearrange("(c p) j -> p c j", p=P))
            amt = wp.tile([P, T2], f32)
            nc.sync.dma_start(
                out=amt,
                in_=bass.AP(tensor=d_am.tensor, offset=d_am.offset,
                            ap=[[0, P], d_am.ap[1]]))
            nc.sync.dma_start(
                out=pri[:, 3:6],
                in_=d_prior[HWD:T1].rearrange("(c p) j -> p c j", p=P))

            # PE warm-up while input DMAs are in flight (HAM clock ramp)
            wrm = wp.tile([P, P], f32)
            nc.vector.memset(wrm, 0.0)
            pwarm = psp.tile([P, P], f32, tag="big")
            for _ in range(6):
                nc.tensor.matmul(pwarm, wrm, wrm, start=True, stop=True)

            # aq row 64 pairs with akt's |k|^2 row
            aq = qp.tile([NA + 1, T1], bf16)
            nc.vector.memset(aq[NA : NA + 1, :], -5e-4)

            # ---- key depthwise conv (k=3): fused mul-add chain on DVE ----
            m0 = kp.tile([P, KC, T2], f32)
            t1t = kp.tile([P, KC, T2], f32)
            kdf = kp.tile([P, KC, T2], bf16)
            for cc in range(KC):
                cw = COL_KDW + 3 * cc
                nc.vector.tensor_scalar_mul(
                    out=m0[:, cc], in0=kpad[:, cc, 0:T2],
                    scalar1=wts[:, cw : cw + 1])
                nc.vector.scalar_tensor_tensor(
                    out=t1t[:, cc], in0=kpad[:, cc, 2 : T2 + 2],
                    scalar=wts[:, cw + 2 : cw + 3], in1=m0[:, cc],
                    op0=ALU.mult, op1=ALU.add)
                nc.vector.scalar_tensor_tensor(
                    out=kdf[:, cc], in0=kpad[:, cc, 1 : T2 + 1],
                    scalar=wts[:, cw + 1 : cw + 2], in1=t1t[:, cc],
                    op0=ALU.mult, op1=ALU.add)

            # ---- key pointwise 256 -> 512 (bf16), two 2-chunk PSUM waves
            pk1h = [psp.tile([P, 2, 512], f32, tag="big", name=f"pk1{w}")
                    for w in range(2)]
            for oc in range(2):
                for cc in range(KC):
                    nc.tensor.matmul(
                        pk1h[0][:, oc, 0:T2],
                        wtb[:, COL_W1K + 512 * cc + P * oc :
                               COL_W1K + 512 * cc + P * (oc + 1)],
                        kdf[:, cc],
                        start=(cc == 0), stop=(cc == KC - 1))

            # ---- query conv1+pointwise 15->30 (f32r, full-rate at 384 cols)
            pq1 = psp.tile([64, 2, 512], f32, tag="q", name="pq1")
            for h in range(2):
                nc.tensor.matmul(
                    pq1[32 * h : 32 * h + 32, 0, 0:HWD],
                    wtb[0:45, COL_Q1W : COL_Q1W + 32],
                    qp3[:, h * HWD : (h + 1) * HWD],
                    start=True, stop=True)

            for oc in range(2, OC):
                for cc in range(KC):
                    nc.tensor.matmul(
                        pk1h[1][:, oc - 2, 0:T2],
                        wtb[:, COL_W1K + 512 * cc + P * oc :
                               COL_W1K + 512 * cc + P * (oc + 1)],
                        kdf[:, cc],
                        start=(cc == 0), stop=(cc == KC - 1))

            # ---- key silu + pointwise 512 -> 64 (bf16), pipelined per oc
            x2k = kp.tile([P, OC, T2], bf16)
            pk2 = psp.tile([NA, T2], f32, tag="k2")
            last_silu = []

            def ksilu_oc(oc):
                pk1s = pk1h[oc // 2][:, oc % 2, 0:T2]
                last_silu.append(nc.scalar.activation(
                    out=x2k[:, oc], in_=pk1s, func=AF.Silu,
                    bias=wts[:, COL_KPB + oc : COL_KPB + oc + 1]))
                nc.tensor.matmul(
                    pk2, wtb[:, COL_W2K + 64 * oc : COL_W2K + 64 * (oc + 1)],
                    x2k[:, oc],
                    start=(oc == 0), stop=(oc == OC - 1))

            ksilu_oc(0)
            ksilu_oc(1)

            # ---- query silu 1 + pointwise 30 -> 15 (stacked halves)
            x2q1 = qp.tile([64, HWD], bf16)
            last_silu.append(nc.scalar.activation(
                out=x2q1, in_=pq1[:, 0, 0:HWD], func=AF.Silu,
                bias=wts[0:64, COL_Q1PB : COL_Q1PB + 1]))
            pq2 = psp.tile([64, 2, 512], f32, tag="q", name="pq2")
            for h in range(2):
                nc.tensor.matmul(pq2[32 * h : 32 * h + 32, 0, 0:HWD],
                                 wtb[32 * h : 32 * h + 30,
                                     COL_Q2W : COL_Q2W + 32],
                                 x2q1[32 * h : 32 * h + 30, :],
                                 start=True, stop=True)

            ksilu_oc(2)
            ksilu_oc(3)

            # ---- query silu 2 + pointwise 15 -> 64
            x2q2 = qp.tile([64, HWD], bf16)
            last_silu.append(nc.scalar.activation(
                out=x2q2, in_=pq2[:, 0, 0:HWD], func=AF.Silu,
                bias=wts[0:64, COL_Q2PB : COL_Q2PB + 1]))
            pq3 = psp.tile([NA, 2, 512], f32, tag="q", name="pq3")
            for h in range(2):
                nc.tensor.matmul(pq3[:, h, 0:HWD],
                                 wtb[32 * h : 32 * h + NS,
                                     COL_Q3W : COL_Q3W + 64],
                                 x2q2[32 * h : 32 * h + NS, :],
                                 start=True, stop=True)

            # ---- augmented ke (65, 192): rows 0..63 ke+bias, row 64 = |k|^2
            akt = kp.tile([NA + 1, T2], bf16)
            nc.vector.tensor_scalar_add(
                out=akt[0:NA, :], in0=pk2,
                scalar1=wts[0:NA, COL_K2PB : COL_K2PB + 1])
            sqk = kp.tile([NA, T2], bf16)
            nc.vector.tensor_mul(out=sqk, in0=akt[0:NA, :], in1=akt[0:NA, :])
            pksq = psp.tile([1, T2], f32, tag="k2", name="pksq")
            nc.tensor.matmul(pksq, wtb[0:NA, COL_ONE : COL_ONE + 1],
                             sqk, start=True, stop=True)
            nc.vector.tensor_copy(out=akt[NA : NA + 1, :], in_=pksq)

            # aq rows 0..63 = 1e-3*qe + 1e-3*q3_pb; h0 on ACT (pre-switch),
            # h1 on DVE (so it cannot delay the exp chain on ACT)
            _aqh0 = nc.scalar.activation(
                out=aq[0:NA, 0:HWD], in_=pq3[:, 0, 0:HWD], func=AF.Identity,
                scale=1e-3, bias=wts[0:NA, COL_Q3PBS : COL_Q3PBS + 1])
            nc.vector.tensor_scalar(
                out=aq[0:NA, HWD:T1], in0=pq3[:, 1, 0:HWD],
                scalar1=1e-3,
                scalar2=wts[0:NA, COL_Q3PBS : COL_Q3PBS + 1],
                op0=ALU.mult, op1=ALU.add)

            # pm = (prior + 1e-8) * mask01, f32
            amb = bass.AP(tensor=amt.tensor, offset=amt.offset,
                          ap=[amt.ap[0], [0, 3], amt.ap[1]])
            pm = sm.tile([P, IC, T2], f32)
            for h in range(2):
                nc.vector.scalar_tensor_tensor(
                    out=pm[:, 3 * h : 3 * h + 3], in0=pri[:, 3 * h : 3 * h + 3],
                    scalar=wts[:, COL_EPS : COL_EPS + 1], in1=amb,
                    op0=ALU.add, op1=ALU.mult)

            # switch ACT to the exp+ln table after the last Silu
            _eld = mybir.InstLoadActFuncSet(
                name=nc.get_next_instruction_name(), ins=[], outs=[],
                act_func_set_id=tabs.index("natural_log_exp_and_others"))
            nc.scalar.add_instruction(_eld)
            tile.add_dep_helper(_eld, last_silu[-1].ins, sync=False,
                                reason="exp table after last silu")
            tile.add_dep_helper(_eld, _aqh0.ins, sync=False,
                                reason="keep aq_h0 before the table switch")

            # ---- attention + softmaxes, chunk-pipelined ------------------
            def mk(nm, shape, dt=f32):
                return [sm.tile(shape, dt, tag=f"{nm}{h}", name=f"{nm}{h}")
                        for h in range(2)]
            z1h = mk("z1", [P, 3])
            lzh = mk("lz", [P, 3])
            r2h = mk("r2", [P, 3])
            z2h = mk("z2", [P, 3])
            oa1 = mk("oa1", [P, 3, T2])
            oa2 = mk("oa2", [P, 3, T2])
            e2t = mk("e2", [P, 3, T2])
            e1t = mk("e1", [P, 3, T2])
            lpp = sm.tile([P, IC, T2], f32)

            pscs = []
            _e1i = None
            for c in range(IC):
                h, col = c // 3, c % 3
                psc = psB.tile([P, T2], f32, tag="ps", name=f"ps{c}")
                pscs.append(psc)
                nc.tensor.matmul(psc, aq[:, c * P : (c + 1) * P], akt,
                                 start=True, stop=True)
                _e1i = nc.scalar.activation(
                    out=e1t[h][:, col, :], in_=psc, func=AF.Exp,
                    bias=wts[:, COL_ZERO : COL_ZERO + 1],
                    accum_out=z1h[h][:, col : col + 1])
                nc.vector.scalar_tensor_tensor(
                    out=e2t[h][:, col, :], in0=e1t[h][:, col, :],
                    scalar=1.0, in1=pm[:, c, :],
                    op0=ALU.mult, op1=ALU.mult,
                    accum_out=z2h[h][:, col : col + 1])
                nc.vector.reciprocal(out=r2h[h][:, col : col + 1],
                                     in_=z2h[h][:, col : col + 1])
                nc.gpsimd.tensor_scalar_mul(
                    out=oa2[h][:, col, :], in0=e2t[h][:, col, :],
                    scalar1=r2h[h][:, col : col + 1])

                if col == 2:
                    # half finalize: lz, then alp = (s - lz) + ln(prior+eps)
                    _lni = nc.scalar.activation(out=lzh[h], in_=z1h[h],
                                                func=AF.Ln)
                    tile.add_dep_helper(_lni.ins, _e1i.ins, sync=False,
                                        reason="ln after exps (table set)")
                    _lpi = nc.scalar.activation(
                        out=lpp[:, 3 * h : 3 * h + 3, :],
                        in_=pri[:, 3 * h : 3 * h + 3, :], func=AF.Ln,
                        bias=wts[:, COL_EPS : COL_EPS + 1])
                    tile.add_dep_helper(_lpi.ins, _e1i.ins, sync=False,
                                        reason="ln after exps (table set)")
                    for cb in range(3 * h, 3 * h + 3):
                        nc.vector.scalar_tensor_tensor(
                            out=oa1[h][:, cb - 3 * h, :], in0=pscs[cb],
                            scalar=lzh[h][:, cb - 3 * h : cb - 3 * h + 1],
                            in1=lpp[:, cb, :],
                            op0=ALU.subtract, op1=ALU.add)
                    rows = slice(h * 3 * P, (h + 1) * 3 * P)
                    nc.sync.dma_start(
                        out=d_attn[rows, :].rearrange("(c p) j -> p c j", p=P),
                        in_=oa2[h])
                    nc.sync.dma_start(
                        out=d_alp[rows, :].rearrange("(c p) j -> p c j", p=P),
                        in_=oa1[h])

    nc.finalize()
    return nc


def _get_nc():
    if "nc" not in _CACHE:
        _CACHE["nc"] = _build()
    return _CACHE["nc"]


def kernel(**inputs):
    from concourse.bass_utils import run_bass_kernel_spmd

    i = {k: np.ascontiguousarray(np.asarray(v)) for k, v in inputs.items()}
    ws, wb = _pack_weights(i)

    in_maps = []
    for b in range(N_CORES):
        in_maps.append({
            "qT": np.ascontiguousarray(i["queries"][b].T),
            "kT": np.ascontiguousarray(i["keys"][b].T),
            "prior": np.ascontiguousarray(i["attn_prior"][b]),
            "am": (~i["mask"][b]).astype(np.float32),
            "wts": ws,
            "wtb": wb,
        })

    nc = _get_nc()
    res = run_bass_kernel_spmd(nc, in_maps, core_ids=list(range(N_CORES)),
                               **_CACHE.get("run_kwargs", {}))
    _CACHE["last_result"] = res

    attn = np.stack([r["attn_out"] for r in res.results])[:, None]
    alp = np.stack([r["alp_out"] for r in res.results])[:, None]
    return attn, alp
